# revision 36
# baseline (speedup 1.0000x reference)
"""2-layer GATConv (PyG-style, edge_dim, self-loops fill='mean') on 8 TRN2 NeuronCores.

Strategy:
  - Host does index-only preprocessing: degree-balanced assignment of nodes to
    8 cores x TPC tiles x 128 slots, per-tile padded edge streams grouped by
    source gid chunk (int16 gather range), wrapped int16 index arrays. No
    feature arithmetic happens on host.
  - Device (per core, SPMD):
      phase0: xh|a_src|a_dst = xT_tile.T @ [W | W@Asrc_bd | W@Adst_bd]  (PE);
              xh rows -> bf16 gather table; a_dst rows -> padded local table.
      AllGather the xh table across 8 cores.
      phase1 (per dst-tile): dma_gather per-edge xh rows (4 src-chunk gathers)
              + per-edge a_dst rows (1 gather from the local table);
              a_edge via packed blockdiag matmul; a_src recomputed from the
              gathered xh on DVE; alpha = leaky_relu(sum); p = exp(alpha)
              (no segment max: softmax is shift-invariant, logits are small);
              segment sums of [p*xh | p | a_edge] via one-hot matmul -> PSUM.
      phase2: analytic self-loop (fill='mean' loop attr = mean of incident
              a_edge), normalize, bias (+relu, layer 1), PE-transpose h for
              the next layer's stationary operand.
"""

import os
import sys

sys.path.insert(0, "/opt/trn_rl_repo")

import numpy as np
import ml_dtypes

import concourse.bass as bass
import concourse.mybir as mybir
from concourse import bacc, tile
from concourse.bass_utils import run_bass_kernel_spmd
from concourse.masks import make_identity

F32 = mybir.dt.float32
BF16 = mybir.dt.bfloat16
I16 = mybir.dt.int16
BF = ml_dtypes.bfloat16

NCORES = 8
H = 4
CH = 32          # channels per head
F = 128          # hidden/out features
ED = 16          # edge feature dim
MROW = 136       # msg/acc row: 128 weighted-msg + 4 p + 4 a_edge
TROW = 132       # table row elems used: 128 xh + 4 a_src
TSTRIDE = 256    # table row stride in elems (512B; DMA stride must be %256B)
SENT = 512.0     # dst-slot sentinel for padded edges
CHUNK = 32768    # int16 gather range


def _wrap16(arr):
    """[NT, L] -> wrapped idx layout [NT, 128, L//16] (replicated x8)."""
    NT, L = arr.shape
    w = arr.reshape(NT, L // 16, 16).transpose(0, 2, 1)     # [NT,16,L/16]
    return np.ascontiguousarray(np.tile(w, (1, 8, 1)))      # [NT,128,L/16]


# --------------------------------------------------------------------------
# host-side index preprocessing
# --------------------------------------------------------------------------

def _preprocess(x, src, dst, edge_attr):
    N = x.shape[0]
    E = src.shape[0]
    TPC = -(-N // (NCORES * 128))          # tiles per core
    NT = NCORES * TPC                      # total tiles
    NPC = TPC * 128                        # padded nodes per core
    NTOT = NT * 128
    NCHK = -(-NTOT // CHUNK)

    deg = np.bincount(dst, minlength=N)

    # snake-deal nodes (sorted by degree desc) into NT tiles of 128 slots
    order = np.argsort(-deg, kind="stable")
    perm = np.full((NT, 128), -1, dtype=np.int64)
    for r in range(128):
        chunk = order[r * NT:(r + 1) * NT]
        tiles = np.arange(len(chunk))
        if r % 2 == 1:
            tiles = NT - 1 - tiles
        perm[tiles, r] = chunk

    flat = perm.reshape(-1)
    valid = flat >= 0
    gid = np.full(N, -1, dtype=np.int64)
    gid[flat[valid]] = np.arange(NTOT)[valid]
    assert (gid >= 0).all()

    d_gid = gid[dst]
    tile_e = d_gid // 128
    slot_e = d_gid % 128
    gid_src = gid[src]
    c4 = gid_src // CHUNK

    # group each tile's edges by src chunk, uniform group lengths
    key = tile_e * NCHK + c4
    korder = np.argsort(key, kind="stable")
    cnt2 = np.bincount(key, minlength=NT * NCHK).reshape(NT, NCHK)
    Lg = (128 * np.ceil(cnt2.max(axis=0) / 128)).astype(np.int64)
    Goff = np.concatenate([[0], np.cumsum(Lg)])
    S = int(Goff[-1])
    G = S // 128
    gstart = np.zeros(NT * NCHK + 1, dtype=np.int64)
    gstart[1:] = np.cumsum(cnt2.reshape(-1))
    pos_in_grp = np.arange(E) - gstart[key[korder]]
    streampos = Goff[c4[korder]] + pos_in_grp
    ET = np.full((NT, S), -1, dtype=np.int64)
    ET[tile_e[korder], streampos] = korder
    ev = ET >= 0
    ETs = np.where(ev, ET, 0)

    # xh gather idx16 (per-group rebased), wrapped per group then concat
    chunk_of_pos = np.repeat(np.arange(NCHK), Lg)            # [S]
    i16 = np.where(ev, gid_src[ETs] - chunk_of_pos[None, :] * CHUNK, 0)
    idx_parts = [
        _wrap16(i16[:, Goff[g]:Goff[g + 1]].astype(np.int16))
        for g in range(NCHK) if Lg[g] > 0
    ]
    idxs = np.concatenate(idx_parts, axis=2)                 # [NT,128,S/16]

    # a_dst gather idx16 (whole stream, single gather)
    ado = np.where(ev, (tile_e[ETs] % TPC) * 128 + slot_e[ETs], 0)
    adofs = _wrap16(ado.astype(np.int16))                    # [NT,128,S/16]

    dl = np.where(ev, slot_e[ETs], int(SENT)).astype(np.float32)
    dl3 = dl.reshape(NT, G, 128)
    dstloc = np.ascontiguousarray(dl3.transpose(0, 2, 1)).astype(BF)

    Q = -(-G // 8)
    GP = Q * 8
    ET3p = np.full((NT, GP, 128), -1, dtype=np.int64)
    ET3p[:, :G] = ET.reshape(NT, G, 128)
    evp = ET3p >= 0
    ea = np.where(evp[..., None], edge_attr[np.where(evp, ET3p, 0)], 0.0)
    # [NT, GP, 128, ED] = [t, k=8q+b, e', c] -> [t, partition=16b+c, q, e']
    eaTp = np.ascontiguousarray(
        ea.reshape(NT, Q, 8, 128, ED).transpose(0, 2, 4, 1, 3).reshape(NT, 128, Q, 128)
    ).astype(BF)

    deg_slot = np.where(perm >= 0, deg[np.where(perm >= 0, perm, 0)], 0)
    cntinv = (1.0 / np.maximum(deg_slot, 1)).astype(np.float32)  # [NT,128]

    xts = []
    for c in range(NCORES):
        pc = perm[c * TPC:(c + 1) * TPC].reshape(-1)
        xp = np.zeros((NPC, F), dtype=np.float32)
        m = pc >= 0
        xp[m] = x[pc[m]]
        xts.append(np.ascontiguousarray(xp.T))   # [F, NPC]

    chunk_lens = [min(CHUNK, NTOT - g * CHUNK) for g in range(NCHK)]
    return dict(N=N, E=E, TPC=TPC, NT=NT, NPC=NPC, NTOT=NTOT, G=G, S=S, Q=Q,
                NCHK=NCHK, Lg=[int(v) for v in Lg], chunk_lens=chunk_lens,
                perm=perm, idxs=idxs, adofs=adofs, dstloc=dstloc,
                eaTp=eaTp, cntinv=cntinv, xts=xts)


def _blockdiag(att):
    """att [H, CH] -> [F, H] block diagonal."""
    out = np.zeros((F, H), dtype=np.float32)
    for h in range(H):
        out[h * CH:(h + 1) * CH, h] = att[h]
    return out


def _raw_dma_gather(gp, out_ap, in_ap, idxs_ap, num_idxs, elem_size,
                    elem_step=None):
    """bass dma_gather minus the transpose-only elem_size%256 assert,
    single_packet=False (lifts the 1024-index cap). HW-validated."""
    from concourse import ap_utils
    from concourse._compat import exact_div
    assert idxs_ap.dtype == mybir.dt.int16
    assert in_ap.dtype == out_ap.dtype
    if elem_step is None:
        assert ap_utils.ap_is_contiguous(in_ap.ap[1:])
        elem_step = elem_size
    assert ap_utils.ap_is_contiguous(out_ap.ap[1:])
    assert ap_utils.ap_is_contiguous(idxs_ap.ap[1:])
    assert in_ap.ap[-1][1] == out_ap.ap[-1][1] == elem_size
    assert in_ap.ap[0][0] == elem_step
    stride_bytes = elem_step * mybir.dt.size(in_ap.dtype)
    stride_bytes_256 = exact_div(stride_bytes, 256)
    _in_ap = gp.lower_ap_dma(in_ap, for_custom_bir_dma=True)
    _idxs_ap = gp.lower_ap(idxs_ap)
    _out_ap = gp.lower_ap(out_ap)
    return gp.add_instruction(
        mybir.InstDMAGatherAnt(
            name=gp.bass.get_next_instruction_name(),
            ins=[*_in_ap, _idxs_ap,
                 gp.lower_val_access(gp.to_reg(num_idxs))],
            outs=[_out_ap],
            transpose=False, num_idxs=num_idxs, elem_size=elem_size,
            stride_bytes_256=stride_bytes_256, gen_mode=0,
            single_packet=False, queue_num=0,
            sbuf_tokens_per_rank=0, sbuf_free_dim_per_rank=0,
            sbuf_free_dim_pad_per_rank=0, sbuf_byte_offset=0,
        ))


# --------------------------------------------------------------------------
# device program
# --------------------------------------------------------------------------

def _build(meta):
    TPC, G, S, Q, NPC, NTOT, NCHK = (
        meta[k] for k in ("TPC", "G", "S", "Q", "NPC", "NTOT", "NCHK"))
    Lg, chunk_lens = meta["Lg"], meta["chunk_lens"]
    S16 = S // 16

    nc = bacc.Bacc("TRN2", target_bir_lowering=False, debug=False,
                   num_devices=NCORES)

    def din(name, shape, dt):
        return nc.dram_tensor(name, list(shape), dt, kind="ExternalInput")

    xT_d = din("xT", (F, NPC), F32)
    idxs_d = din("idxs", (TPC, 128, S16), I16)
    adofs_d = din("adofs", (TPC, 128, S16), I16)
    dstloc_d = din("dstloc", (TPC, 128, G), BF16)
    eaTp_d = din("eaTp", (TPC, 128, Q, 128), BF16)
    cntinv_d = din("cntinv", (TPC, 128, 1), F32)
    Wp = [din(f"W{l}", (F, F), F32) for l in (1, 2)]
    WTp = [din(f"WT{l}", (F, F), F32) for l in (1, 2)]
    Asdp = [din(f"Asd{l}", (F, 2 * H), F32) for l in (1, 2)]
    Aep = [din(f"Ae{l}", (F, H), F32) for l in (1, 2)]
    WeTp = [din(f"WeT{l}", (F, ED), F32) for l in (1, 2)]
    biasp = [din(f"b{l}", (1, F), F32) for l in (1, 2)]
    out_d = nc.dram_tensor("out", [NPC, F], F32, kind="ExternalOutput")

    adtab_d = nc.dram_tensor("adtab", [NPC, 128], BF16)
    ltab_d = nc.dram_tensor("ltab", [NPC, TSTRIDE], BF16)
    gtab_d = nc.dram_tensor("gtab", [NTOT, TSTRIDE], BF16, addr_space="Shared")
    ltab2_d = nc.dram_tensor("ltab2", [NPC, TSTRIDE], BF16)
    gtab2_d = nc.dram_tensor("gtab2", [NTOT, TSTRIDE], BF16, addr_space="Shared")
    hT_d = nc.dram_tensor("hT", [F, NPC], F32)

    rg = [list(range(NCORES))]

    with tile.TileContext(nc) as tc:
        with (
            tc.tile_pool(name="persist", bufs=1) as pp,
            tc.tile_pool(name="sb", bufs=2) as sb,
            tc.tile_pool(name="sbg", bufs=3) as sbg,
            tc.tile_pool(name="ps", bufs=2, space="PSUM") as ps,    # ph0/setup, acc
            tc.tile_pool(name="ps1", bufs=2, space="PSUM") as ps1,  # pae
        ):
            # ---- one-time constants ----
            ident = pp.tile([128, 128], F32)
            make_identity(nc, ident[:])
            iota_i = pp.tile([128, 128], mybir.dt.int32, tag="ioti")
            nc.gpsimd.iota(iota_i[:], pattern=[[1, 128]], base=0,
                           channel_multiplier=0)
            iotaRow = pp.tile([128, 128], BF16)
            nc.vector.tensor_copy(iotaRow[:], iota_i[:])

            asrc_all = pp.tile([128, TPC, H], F32)
            adst_all = pp.tile([128, TPC, H], F32)
            ones_sb = pp.tile([1, 128], F32, tag="ones")
            nc.vector.memset(ones_sb[:], 1.0)
            bias_full = pp.tile([128, F], F32, tag="biasf")
            bias_sb = pp.tile([1, F], F32, tag="bias")

            for li in range(2):
                layer1 = li == 0
                ltab = ltab_d if layer1 else ltab2_d
                gtab = gtab_d if layer1 else gtab2_d

                # ---- layer weight prep ----
                wet_sb = sb.tile([F, ED], F32, tag="wet_sb")
                nc.sync.dma_start(out=wet_sb[:], in_=WeTp[li][:, :])
                ae_sb = sb.tile([F, H], F32, tag="ae_sb")
                nc.sync.dma_start(out=ae_sb[:], in_=Aep[li][:, :])
                wae_ps = ps.tile([ED, H], F32, tag="ph0")
                nc.tensor.matmul(out=wae_ps[:], lhsT=wet_sb[:],
                                 rhs=ae_sb[:], start=True, stop=True)
                wae_sb = sb.tile([ED, H], BF16, tag="wae_sb")
                nc.vector.tensor_copy(wae_sb[:], wae_ps[:])
                wae_rep = sb.tile([128, 32], BF16, tag="wae_rep")
                nc.vector.memset(wae_rep[:], 0)
                for b in range(8):
                    nc.gpsimd.dma_start(
                        out=wae_rep[16 * b:16 * b + 16, 4 * b:4 * b + 4],
                        in_=wae_sb[:])

                wt_sb = sb.tile([F, F], F32, tag="wt_sb")
                nc.sync.dma_start(out=wt_sb[:], in_=WTp[li][:, :])
                asd_sb = sb.tile([F, 2 * H], F32, tag="asd_sb")
                nc.sync.dma_start(out=asd_sb[:], in_=Asdp[li][:, :])
                wasd_ps = ps.tile([F, 2 * H], F32, tag="ph0")
                nc.tensor.matmul(out=wasd_ps[:], lhsT=wt_sb[:],
                                 rhs=asd_sb[:], start=True, stop=True)
                wcomb = sb.tile([F, F + 2 * H], F32, tag="wcomb")
                nc.sync.dma_start(out=wcomb[:, 0:F], in_=Wp[li][:, :])
                nc.vector.tensor_copy(wcomb[:, F:F + 2 * H], wasd_ps[:])

                nc.sync.dma_start(out=bias_sb[:], in_=biasp[li][:, :])
                bias_ps = ps.tile([128, F], F32, tag="ph0")
                nc.tensor.matmul(out=bias_ps[:], lhsT=ones_sb[:],
                                 rhs=bias_sb[:], start=True, stop=True)
                nc.vector.tensor_copy(bias_full[:], bias_ps[:])

                # ---- phase 0: xh | a_src | a_dst per node tile ----
                for t in range(TPC):
                    xt = sb.tile([128, 128], F32, tag="xt")
                    src_slab = xT_d if layer1 else hT_d
                    nc.sync.dma_start(out=xt[:],
                                      in_=src_slab[:, t * 128:(t + 1) * 128])
                    ph0 = ps.tile([128, F + 2 * H], F32, tag="ph0")
                    nc.tensor.matmul(out=ph0[:], lhsT=xt[:], rhs=wcomb[:],
                                     start=True, stop=True)
                    tab = sb.tile([128, TROW], BF16, tag="tab")
                    nc.vector.tensor_copy(tab[:], ph0[:, 0:TROW])
                    nc.sync.dma_start(out=ltab[t * 128:(t + 1) * 128, 0:TROW],
                                      in_=tab[:])
                    nc.vector.tensor_copy(asrc_all[:, t, :], ph0[:, F:F + H])
                    nc.vector.tensor_copy(adst_all[:, t, :],
                                          ph0[:, F + H:F + 2 * H])
                    adrow = sb.tile([128, H], BF16, tag="adrow")
                    nc.vector.tensor_copy(adrow[:], ph0[:, F + H:F + 2 * H])
                    nc.scalar.dma_start(out=adtab_d[t * 128:(t + 1) * 128, 0:H],
                                        in_=adrow[:])

                # ---- all-gather the xh table ----
                nc.gpsimd.collective_compute(
                    "AllGather", mybir.AluOpType.bypass, replica_groups=rg,
                    ins=[ltab[:, :].opt()], outs=[gtab[:, :].opt()])

                # ---- phase 1/2 per dst tile ----
                for t in range(TPC):
                    idx_t = sb.tile([128, S16], I16, tag="idx")
                    nc.scalar.dma_start(out=idx_t[:], in_=idxs_d[t, :, :])
                    ado_t = sb.tile([128, S16], I16, tag="ado")
                    nc.scalar.dma_start(out=ado_t[:], in_=adofs_d[t, :, :])
                    dl_t = sb.tile([128, G], BF16, tag="dl")
                    nc.scalar.dma_start(out=dl_t[:], in_=dstloc_d[t, :, :])
                    eap_t = sb.tile([128, Q, 128], BF16, tag="eap")
                    nc.sync.dma_start(out=eap_t[:], in_=eaTp_d[t, :, :, :])
                    cinv_t = sb.tile([128, 1], F32, tag="cinv")
                    nc.scalar.dma_start(out=cinv_t[:], in_=cntinv_d[t, :, :])

                    xg = sbg.tile([128, G, TROW], BF16, tag="xg")
                    goff = 0
                    for g in range(NCHK):
                        if Lg[g] == 0:
                            continue
                        _raw_dma_gather(
                            nc.gpsimd,
                            xg[:, goff // 128:(goff + Lg[g]) // 128, :],
                            gtab[g * CHUNK:g * CHUNK + chunk_lens[g], 0:TROW],
                            idx_t[:, goff // 16:(goff + Lg[g]) // 16],
                            Lg[g], TROW, elem_step=TSTRIDE)
                        goff += Lg[g]
                    adg = sbg.tile([128, G, H], BF16, tag="adg")
                    _raw_dma_gather(nc.gpsimd, adg[:], adtab_d[:, 0:H],
                                    ado_t[:], S, H, elem_step=128)

                    ind = sbg.tile([128, G, 128], BF16, tag="ind")
                    nc.vector.tensor_tensor(
                        out=ind[:],
                        in0=dl_t[:].unsqueeze(2).to_broadcast([128, G, 128]),
                        in1=iotaRow[:].unsqueeze(1).to_broadcast([128, G, 128]),
                        op=mybir.AluOpType.is_equal)

                    pae = ps1.tile([128, Q, 32], F32, tag="pae")
                    for q in range(Q):
                        nc.tensor.matmul(out=pae[:, q, :], lhsT=eap_t[:, q, :],
                                         rhs=wae_rep[:], start=True, stop=True)
                    pav = pae[:].rearrange("p q (b h) -> p (q b) h", h=H)

                    z = sb.tile([128, G, H], F32, tag="z")
                    nc.vector.tensor_add(z[:], pav[:, 0:G, :],
                                         xg[:, :, F:TROW])
                    nc.vector.tensor_add(z[:], z[:], adg[:])
                    zl = sb.tile([128, G, H], F32, tag="zl")
                    nc.vector.tensor_scalar_mul(zl[:], z[:], 0.2)
                    nc.vector.tensor_max(z[:], z[:], zl[:])
                    p_t = sb.tile([128, G, H], F32, tag="p")
                    nc.scalar.activation(out=p_t[:], in_=z[:],
                                         func=mybir.ActivationFunctionType.Exp)

                    msgstat = sbg.tile([128, G, MROW], BF16, tag="msgstat")
                    nc.vector.tensor_tensor(
                        out=msgstat[:, :, 0:F].rearrange(
                            "p g (h c) -> p g h c", c=CH),
                        in0=xg[:, :, 0:F].rearrange("p g (h c) -> p g h c", c=CH),
                        in1=p_t[:].unsqueeze(3).to_broadcast([128, G, H, CH]),
                        op=mybir.AluOpType.mult)
                    nc.vector.tensor_copy(msgstat[:, :, F:F + H], p_t[:])
                    nc.vector.tensor_copy(msgstat[:, :, F + H:MROW],
                                          pav[:, 0:G, :])

                    acc = ps.tile([128, MROW], F32, tag="acc")
                    for k in range(G):
                        nc.tensor.matmul(out=acc[:], lhsT=ind[:, k, :],
                                         rhs=msgstat[:, k, :],
                                         start=(k == 0), stop=(k == G - 1))

                    # ---- phase 2: self loop + normalize ----
                    selfrow = sb.tile([128, F], BF16, tag="selfrow")
                    nc.scalar.dma_start(out=selfrow[:],
                                      in_=ltab[t * 128:(t + 1) * 128, 0:F])
                    sl = sb.tile([128, 3 * H], F32, tag="sl")
                    # [0:H] = alpha_loop work, [H:2H] = p_loop, [2H:3H] = 1/s
                    nc.vector.tensor_scalar_mul(sl[:, 0:H],
                                                acc[:, F + H:MROW], cinv_t[:])
                    nc.vector.tensor_add(sl[:, 0:H], sl[:, 0:H],
                                         asrc_all[:, t, :])
                    nc.vector.tensor_add(sl[:, 0:H], sl[:, 0:H],
                                         adst_all[:, t, :])
                    nc.vector.tensor_scalar_mul(sl[:, H:2 * H], sl[:, 0:H], 0.2)
                    nc.vector.tensor_max(sl[:, 0:H], sl[:, 0:H], sl[:, H:2 * H])
                    nc.scalar.activation(out=sl[:, H:2 * H], in_=sl[:, 0:H],
                                         func=mybir.ActivationFunctionType.Exp)
                    nc.vector.tensor_add(sl[:, 2 * H:3 * H], acc[:, F:F + H],
                                         sl[:, H:2 * H])
                    nc.vector.tensor_scalar_add(sl[:, 2 * H:3 * H],
                                                sl[:, 2 * H:3 * H], 1e-16)
                    nc.vector.reciprocal(sl[:, 2 * H:3 * H], sl[:, 2 * H:3 * H])

                    of = sb.tile([128, F], F32, tag="of")
                    of4 = of[:].rearrange("p (h c) -> p h c", c=CH)
                    nc.vector.tensor_tensor(
                        out=of4,
                        in0=selfrow[:].rearrange("p (h c) -> p h c", c=CH),
                        in1=sl[:, H:2 * H].unsqueeze(2).to_broadcast(
                            [128, H, CH]),
                        op=mybir.AluOpType.mult)
                    nc.vector.tensor_add(of[:], of[:], acc[:, 0:F])
                    nc.vector.tensor_tensor(
                        out=of4, in0=of4,
                        in1=sl[:, 2 * H:3 * H].unsqueeze(2).to_broadcast(
                            [128, H, CH]),
                        op=mybir.AluOpType.mult)
                    nc.vector.tensor_add(of[:], of[:], bias_full[:])

                    if layer1:
                        nc.vector.tensor_scalar_max(of[:], of[:], 0.0)
                        trp = ps.tile([128, 128], F32, tag="ph0")
                        nc.tensor.transpose(out=trp[:], in_=of[:],
                                            identity=ident[:])
                        trs = sb.tile([128, 128], F32, tag="trs")
                        nc.vector.tensor_copy(trs[:], trp[:])
                        nc.sync.dma_start(out=hT_d[:, t * 128:(t + 1) * 128],
                                          in_=trs[:])
                    else:
                        nc.sync.dma_start(out=out_d[t * 128:(t + 1) * 128, :],
                                          in_=of[:])

    nc.compile()
    return nc


# --------------------------------------------------------------------------
# entry point
# --------------------------------------------------------------------------

def _make_in_maps(meta, inputs):
    wmaps = {}
    for li in (1, 2):
        W = np.asarray(inputs[f"W{li}"], np.float32)
        wmaps[f"W{li}"] = W
        wmaps[f"WT{li}"] = np.ascontiguousarray(W.T)
        wmaps[f"Asd{li}"] = np.concatenate(
            [_blockdiag(np.asarray(inputs[f"att_src{li}"], np.float32)),
             _blockdiag(np.asarray(inputs[f"att_dst{li}"], np.float32))],
            axis=1)
        wmaps[f"Ae{li}"] = _blockdiag(
            np.asarray(inputs[f"att_edge{li}"], np.float32))
        wmaps[f"WeT{li}"] = np.ascontiguousarray(
            np.asarray(inputs[f"W_edge{li}"], np.float32).T)
        wmaps[f"b{li}"] = np.asarray(
            inputs[f"bias{li}"], np.float32).reshape(1, F)

    TPC = meta["TPC"]
    in_maps = []
    for c in range(NCORES):
        tsl = slice(c * TPC, (c + 1) * TPC)
        m = dict(wmaps)
        m["xT"] = meta["xts"][c]
        m["idxs"] = np.ascontiguousarray(meta["idxs"][tsl])
        m["adofs"] = np.ascontiguousarray(meta["adofs"][tsl])
        m["dstloc"] = np.ascontiguousarray(meta["dstloc"][tsl])
        m["eaTp"] = np.ascontiguousarray(meta["eaTp"][tsl])
        m["cntinv"] = np.ascontiguousarray(
            meta["cntinv"][tsl].reshape(TPC, 128, 1))
        in_maps.append(m)
    return in_maps


def kernel(x, edge_index, edge_attr,
           W1, att_src1, att_dst1, W_edge1, att_edge1, bias1,
           W2, att_src2, att_dst2, W_edge2, att_edge2, bias2):
    x = np.asarray(x, np.float32)
    edge_attr = np.asarray(edge_attr, np.float32)
    src = np.asarray(edge_index[0], np.int64)
    dst = np.asarray(edge_index[1], np.int64)

    import time
    t0 = time.time()
    meta = _preprocess(x, src, dst, edge_attr)
    t1 = time.time()
    nc = _build(meta)
    t2 = time.time()
    print(f"preprocess {t1 - t0:.1f}s  build+compile {t2 - t1:.1f}s "
          f"(G={meta['G']} S={meta['S']} TPC={meta['TPC']})", flush=True)

    inputs = dict(W1=W1, att_src1=att_src1, att_dst1=att_dst1,
                  W_edge1=W_edge1, att_edge1=att_edge1, bias1=bias1,
                  W2=W2, att_src2=att_src2, att_dst2=att_dst2,
                  W_edge2=W_edge2, att_edge2=att_edge2, bias2=bias2)
    in_maps = _make_in_maps(meta, inputs)

    trace = os.environ.get("GNN_TRACE") == "1"
    t3 = time.time()
    res = run_bass_kernel_spmd(nc, in_maps, list(range(NCORES)), trace=trace)
    print(f"run {time.time() - t3:.1f}s", flush=True)
    if trace and res.exec_time_ns is not None:
        print(f"HW exec time: {res.exec_time_ns} ns", flush=True)

    out = np.zeros((meta["N"], F), dtype=np.float32)
    perm = meta["perm"]
    TPC = meta["TPC"]
    for c in range(NCORES):
        oc = np.asarray(res.results[c]["out"], np.float32)
        pc = perm[c * TPC:(c + 1) * TPC].reshape(-1)
        mk = pc >= 0
        out[pc[mk]] = oc[mk]
    return out
